# revision 1
# baseline (speedup 1.0000x reference)
"""MQA attention kernel for Trainium2, 8-core SPMD.

Problem: Q [2, 8, 2048, 64] fp32, K/V [2, 1, 2048, 64] fp32 (shared head).
out[b,h,q,:] = softmax(Q[b,h,q,:] @ K[b,0]^T / 8) @ V[b,0].

Sharding: 16 (b,h) pairs over 8 cores -> core c handles b = c//4,
heads 2*(c%4), 2*(c%4)+1 (both heads share one K/V slice).

Per-core kernel (matmuls fp16, accumulation fp32 in PSUM):
  - One SWDGE cast-DMA packs Q[h0]|Q[h1] as the column halves of an HBM
    scratch [S, 128] fp16; one XBAR transpose-DMA turns that into
    QT [128, S] (partitions 0-63 = head0^T, 64-127 = head1^T).
  - K is cast with a broadcast source into both halves of its scratch ->
    KT [128, S] holds K^T replicated on both partition halves.
  - MM1 (scores^T): per k-tile kt, two matmuls on different PE row
    groups (contract=64) compute S_T[kt] [128k, 512q] for head0 (rows
    0-63) and head1 (rows 64-127) concurrently.
  - ACT exp on the 2-bank PSUM group [128, 2, 512] -> P^T fp16 in SBUF
    (1/sqrt(D) folded into the activation's free affine).
  - MM2: out^T[h] [65, 512] += V_aug[kt]^T @ P^T[kt,h]; V_aug carries a
    65th all-ones column so row 64 accumulates the softmax denominator.
  - PE-transpose out^T 128-column slices -> [128q, 65], DVE reciprocal +
    per-partition tensor_scalar mult to normalize, DMA out.
"""

import numpy as np

import concourse.bass as bass
import concourse.bacc as bacc
import concourse.mybir as mybir
import concourse.tile as tile
from concourse.bass_utils import run_bass_kernel_spmd
from concourse.masks import make_identity

F32 = mybir.dt.float32
F16 = mybir.dt.float16

B, H, S, D = 2, 8, 2048, 64
HPC = 2            # heads per core
NCORES = 8
QB = 512           # query block (PSUM bank free-dim limit for fp32)
NQB = S // QB      # 4
KT_TILE = 128      # keys per k-tile (PE contract partition limit)
NKT = S // KT_TILE # 16
SCALE = 1.0 / np.sqrt(np.float32(D))  # 0.125


def build_nc():
    nc = bacc.Bacc(None)
    Qd = nc.declare_dram_parameter("q", [HPC, S, D], F32, isOutput=False)
    Kd = nc.declare_dram_parameter("k", [S, D], F32, isOutput=False)
    Vd = nc.declare_dram_parameter("v", [S, D], F32, isOutput=False)
    Od = nc.declare_dram_parameter("o", [HPC, S, D], F32, isOutput=True)

    with tile.TileContext(nc) as tc:
        with (
            tc.tile_pool(name="const", bufs=1) as constp,
            tc.tile_pool(name="qk", bufs=1) as qkp,
            tc.tile_pool(name="vt", bufs=1) as vp,
            tc.tile_pool(name="pt", bufs=4) as ptp,
            tc.tile_pool(name="ot", bufs=2) as otp,
            tc.tile_pool(name="outsb", bufs=3) as outp,
            tc.tile_pool(name="rec", bufs=3) as recp,
            tc.tile_pool(name="psS", bufs=2, space="PSUM") as psSp,
            tc.tile_pool(name="psO", bufs=1, space="PSUM") as psOp,
            tc.tile_pool(name="psT", bufs=2, space="PSUM") as psTp,
        ):
            ident = constp.tile([128, 128], F32)
            make_identity(nc, ident[:])
            ident16 = constp.tile([128, 128], F16)
            make_identity(nc, ident16[:])

            # Prime the exp table load so the ~2.7us ACT_TABLE_LOAD overlaps
            # the input DMA phase instead of stalling the first real exp.
            dummy = constp.tile([128, 16], F32)
            nc.vector.memset(dummy[:], 0.0)
            nc.scalar.activation(dummy[:], dummy[:], mybir.ActivationFunctionType.Exp)

            # ---- input staging (all on-chip; transpose-DMA has a 1-wait
            # budget in walrus codegen, so PE-mode transposes are used
            # instead, in the window where PE is idle anyway) ----
            Qn = qkp.tile([128, HPC, NKT, D], F32, name="Qn")
            for h in range(HPC):
                nc.sync.dma_start(
                    out=Qn[:, h, :, :],
                    in_=Qd.ap()[h].rearrange("(t p) d -> p t d", p=128),
                )
            Kn = qkp.tile([128, NKT, D], F32, name="Kn")
            nc.sync.dma_start(
                out=Kn[:], in_=Kd.ap().rearrange("(t p) d -> p t d", p=128)
            )
            Qh = qkp.tile([128, HPC, NKT, D], F16, name="Qh")
            nc.vector.tensor_copy(Qh[:], Qn[:])
            Kh = qkp.tile([128, NKT, D], F16, name="Kh")
            nc.vector.tensor_copy(Kh[:], Kn[:])

            # V tiles [128k, kt, 65] fp16, 65th column = 1.0 (denominator).
            Vt = vp.tile([128, NKT, D + 1], F16)
            nc.gpsimd.dma_start(
                out=Vt[:, :, 0:D],
                in_=Vd.ap().rearrange("(t p) d -> p t d", p=128),
            )
            nc.vector.memset(Vt[:, :, D : D + 1], 1.0)

            # KT [128, S]: K^T on partitions 0-63 via PE transposes, then
            # replicated to 64-127 with one SBUF->SBUF DMA.
            # QT [128, S]: head0^T on partitions 0-63, head1^T on 64-127.
            KT = qkp.tile([128, S], F16, name="KT")
            QT = qkp.tile([128, S], F16, name="QT")
            for t in range(NKT):
                ts_ = slice(t * 128, (t + 1) * 128)
                psk = psTp.tile([64, 128], F16, tag="pst")
                nc.tensor.transpose(psk[:], Kh[:, t, :], ident16[:])
                nc.vector.tensor_copy(KT[0:64, ts_], psk[:])
                psq = psTp.tile([128, 128], F16, tag="pst")
                for h in range(HPC):
                    nc.tensor.transpose(
                        psq[64 * h : 64 * (h + 1), :],
                        Qh[:, h, t, :],
                        ident16[:],
                        tile_position=(0, 64 * h),
                    )
                nc.vector.tensor_copy(QT[:, ts_], psq[:])
            nc.sync.dma_start(out=KT[64:128, :], in_=KT[0:64, :])

            # ---- main loop ----
            for qb in range(NQB):
                qs = slice(qb * QB, (qb + 1) * QB)
                ps_o = [psOp.tile([D + 1, QB], F32, name=f"psO{h}") for h in range(HPC)]
                for kt in range(NKT):
                    ks = slice(kt * KT_TILE, (kt + 1) * KT_TILE)
                    ps_s = psSp.tile([128, HPC, QB], F32)
                    for h in range(HPC):
                        nc.tensor.matmul(
                            ps_s[:, h, :],
                            lhsT=KT[64 * h : 64 * (h + 1), ks],
                            rhs=QT[64 * h : 64 * (h + 1), qs],
                            start=True,
                            stop=True,
                        )
                    pt = ptp.tile([128, HPC, QB], F16)
                    nc.scalar.activation(
                        pt[:],
                        ps_s[:],
                        mybir.ActivationFunctionType.Exp,
                        scale=float(SCALE),
                    )
                    for h in range(HPC):
                        nc.tensor.matmul(
                            ps_o[h][:],
                            lhsT=Vt[:, kt, :],
                            rhs=pt[:, h, :],
                            start=(kt == 0),
                            stop=(kt == NKT - 1),
                        )
                # ---- drain: transpose + normalize + store ----
                for h in range(HPC):
                    ot = otp.tile([D + 1, QB], F32)
                    nc.vector.tensor_copy(ot[:], ps_o[h][:])
                    ps_t = psTp.tile([128, QB // 128, D + 1], F32, tag="pst")
                    rec = recp.tile([128, QB // 128, 1], F32)
                    outsb = outp.tile([128, QB // 128, D], F32)
                    for j in range(QB // 128):
                        nc.tensor.transpose(
                            ps_t[:, j, :],
                            ot[:, j * 128 : (j + 1) * 128],
                            ident[0 : D + 1, 0 : D + 1],
                        )
                        nc.vector.reciprocal(rec[:, j, :], ps_t[:, j, D : D + 1])
                        nc.vector.tensor_scalar_mul(
                            outsb[:, j, :], ps_t[:, j, 0:D], rec[:, j, :]
                        )
                    nc.sync.dma_start(
                        out=Od.ap()[h, qs, :].rearrange("(j p) d -> p j d", p=128),
                        in_=outsb[:],
                    )
    nc.compile()
    return nc


_CACHED = {}


def _get_nc():
    if "nc" not in _CACHED:
        _CACHED["nc"] = build_nc()
    return _CACHED["nc"]


def _shard(Q, K, V):
    in_maps = []
    for c in range(NCORES):
        b = c // 4
        h0 = (c % 4) * HPC
        in_maps.append(
            {
                "q": np.ascontiguousarray(np.asarray(Q, np.float32)[b, h0 : h0 + HPC]),
                "k": np.ascontiguousarray(np.asarray(K, np.float32)[b, 0]),
                "v": np.ascontiguousarray(np.asarray(V, np.float32)[b, 0]),
            }
        )
    return in_maps


def kernel(Q, K, V, trace=False):
    nc = _get_nc()
    res = run_bass_kernel_spmd(nc, _shard(Q, K, V), list(range(NCORES)), trace=trace)
    _CACHED["last_result"] = res
    O = np.empty((B, H, S, D), np.float32)
    for c, r in enumerate(res.results):
        b = c // 4
        h0 = (c % 4) * HPC
        O[b, h0 : h0 + HPC] = r["o"]
    return O

